# revision 1
# baseline (speedup 1.0000x reference)
"""Bass/Trainium2 kernel for nn_DirectedMessagePassingLayer_65807488909810.

Reference computation:
    agg_in  = segment_sum(vals_in[:,None]  * x[cols_in],  rows_in,  n)
    agg_out = segment_sum(vals_out[:,None] * x[cols_out], rows_out, n)
    h = x @ W_self.T + b_self + agg_in @ W_in.T + agg_out @ W_out.T
    out = relu(layernorm(h) * gamma + beta)        # gamma=1, beta=0 handled

Distribution (8 NeuronCores, SPMD — one compiled program, per-core data):
  nodes (rows of x / output) are sharded 6250/core; edges are partitioned by
  destination row so the segment-sum is core-local; x is replicated as a
  row-major gather table; weights/LN params replicated.

Per-core algorithm:
  * Edge slots are laid out on a uniform grid (set, dest-block-of-128,
    dest-window-of-64, source-half) padded to multiples of 128; per-cell
    chunk counts are the max over all cores so all 8 cores share one
    instruction stream.
  * Slots are gathered from the (bf16) x table with InstDMAGatherAnt in
    batches of 128*SB rows (int16 indices force a lo/hi table split).
  * Per batch, a scaled one-hot S[e, j] = val[e] * (rl[e] == j) is built in
    bulk on the vector engine.
  * Per 128-slot chunk the tensor engine scatters into a PSUM accumulator:
        PSUM_agg[feat, dest_win] += G_chunk.T @ S_chunk
  * Per block: hT = WselfT.T @ xT_blk + WinT.T @ aggT_in + WoutT.T @ aggT_out
    accumulated in PSUM, bias added on the PSUM->SBUF copy, PE-transposed,
    layer-normed (free-dim stats) + relu'd, stored.
"""

import numpy as np
import ml_dtypes

import concourse.bass as bass
import concourse.bacc as bacc
import concourse.mybir as mybir
import concourse.tile as tile
from concourse.bass_utils import run_bass_kernel_spmd
from concourse.masks import make_identity

# ---------------- problem constants (hardcoded per contract) ----------------
N_NODES = 50000
D = 128
LN_EPS = 1e-5
N_CORES = 8
ROWS_PER_CORE = 6250
BLOCKS = 49                   # ceil(6250/128)
PAD_ROWS = BLOCKS * 128       # 6272
WIN = 64                      # dest window width
CHUNK = 128
SB = 32                       # gather batch stripes (4096-row gathers)
XLO_ROWS = 32768              # lo table = x[0:XLO_ROWS]
HI_BASE = 17232               # hi table = x[HI_BASE:] (32768 rows)
LO_TARGET = 384               # per-cell lo-stream fill target (3 chunks)

F32 = mybir.dt.float32
BF16 = mybir.dt.bfloat16
I16 = mybir.dt.int16

USE_BF16_GATHER = True


def _split_multi_waits(nc):
    """This walrus build encodes at most one sync-wait per instruction;
    split N-wait instructions into N-1 preceding single-wait NoOps
    (engine-serial execution preserves the semantics)."""
    k = 0
    for f in nc.m.functions:
        for bb in f.blocks:
            new = []
            for inst in bb.instructions:
                si = inst.sync_info
                if si is not None and si.on_wait is not None and len(si.on_wait) > 1:
                    waits = list(si.on_wait)
                    for w in waits[:-1]:
                        k += 1
                        new.append(mybir.InstNoOp(
                            name=f"waitsplit-{k}", engine=inst.engine,
                            ins=[], outs=[],
                            sync_info=mybir.SyncInfo(on_wait=[w], on_update=[])))
                    si.on_wait = waits[-1:]
                new.append(inst)
            bb.instructions = new
    return k


def _wrap_slots(a, nb, sb):
    """[n_slots] -> [128, stripes] with slot g at [g%128, g//128]."""
    return np.ascontiguousarray(a.reshape(nb * sb, 128).T)


def _wrap_idx16(a, nb, sb):
    """[n_slots] -> [128, stripes*16th] in dma_gather's per-batch 16-wrap:
    batch bi, in-batch j -> [j%16 (replicated x8), bi*(sb*8) + j//16]."""
    n_per_batch = sb * 128
    A = a.reshape(nb, n_per_batch // 16, 16)          # [nb, cols, 16]
    B = A.transpose(2, 0, 1).reshape(16, nb * (n_per_batch // 16))
    return np.ascontiguousarray(np.tile(B, (8, 1)))


def _build_layout(edge_sets):
    """Uniform slot layout across cores.

    Returns (prog, per-stream stripes, per-core wrapped arrays).
    prog: list over blocks of [(s, w, c_lo, c_hi)] in program order.
    """
    nsets = len(edge_sets)
    cnt = np.zeros((nsets, N_CORES, BLOCKS, 2, 2), dtype=np.int64)
    fields = []
    for s, (rows, cols, vals) in enumerate(edge_sets):
        core = rows // ROWS_PER_CORE
        rloc = rows - core * ROWS_PER_CORE
        b = rloc >> 7
        w = (rloc >> 6) & 1
        rl = rloc & 63
        cell = (core * BLOCKS + b) * 2 + w
        must_hi = cols >= XLO_ROWS
        flex = (cols >= HI_BASE) & ~must_hi
        ncell = N_CORES * BLOCKS * 2
        m_lo_cell = np.bincount(cell[~must_hi & ~flex], minlength=ncell)
        f_cell = np.bincount(cell[flex], minlength=ncell)
        quota = np.clip(LO_TARGET - m_lo_cell, 0, f_cell)
        # rank of each flex edge within its cell (stable)
        fi = np.flatnonzero(flex)
        fo = fi[np.argsort(cell[fi], kind="stable")]
        fstart = np.concatenate([[0], np.cumsum(f_cell)[:-1]])
        rank = np.empty(len(fo), dtype=np.int64)
        rank[:] = np.arange(len(fo)) - fstart[cell[fo]]
        h = must_hi.astype(np.int64)
        h[fo] = (rank >= quota[cell[fo]]).astype(np.int64)
        key = cell * 2 + h
        order = np.argsort(key, kind="stable")
        c = np.bincount(key, minlength=N_CORES * BLOCKS * 2 * 2)
        cnt[s] = c.reshape(N_CORES, BLOCKS, 2, 2)
        starts = np.concatenate([[0], np.cumsum(c)[:-1]])
        fields.append((order, starts, rl, cols, vals))

    # chunk counts per (s, b, w, h): max over cores
    C = -(-cnt.max(axis=1) // CHUNK)          # [nsets, BLOCKS, 2, 2]
    # guarantee PSUM start coverage per (s, b, w)
    empty = (C.sum(axis=3) == 0)              # [nsets, BLOCKS, 2]
    C[:, :, :, 0] += empty.astype(np.int64)

    n_slots = [int(C[:, :, :, h].sum()) * CHUNK for h in range(2)]
    batch = CHUNK * SB
    nb = [max(1, -(-n // batch)) for n in n_slots]

    idx = [np.zeros((N_CORES, nb[h] * batch), dtype=np.int16) for h in range(2)]
    rl_a = [np.zeros((N_CORES, nb[h] * batch), dtype=np.float32) for h in range(2)]
    val_a = [np.zeros((N_CORES, nb[h] * batch), dtype=np.float32) for h in range(2)]

    prog = []
    pos = [0, 0]
    for b in range(BLOCKS):
        row = []
        for s in range(nsets):
            for w in range(2):
                row.append((s, w, int(C[s, b, w, 0]), int(C[s, b, w, 1])))
                for h in range(2):
                    c = int(C[s, b, w, h])
                    if c == 0:
                        continue
                    for ci in range(N_CORES):
                        order, starts, rl, cols, vals = fields[s]
                        key = ((ci * BLOCKS + b) * 2 + w) * 2 + h
                        st = int(starts[key])
                        n = int(cnt[s, ci, b, w, h])
                        sel = order[st:st + n]
                        p = pos[h]
                        idx[h][ci, p:p + n] = (cols[sel] - h * HI_BASE).astype(np.int16)
                        rl_a[h][ci, p:p + n] = rl[sel]
                        val_a[h][ci, p:p + n] = vals[sel]
                    pos[h] += c * CHUNK
        prog.append(row)
    assert pos[0] == n_slots[0] and pos[1] == n_slots[1]

    out = {"prog": prog, "nb": nb}
    for h in range(2):
        out[f"idx{h}"] = np.stack([_wrap_idx16(idx[h][ci], nb[h], SB)
                                   for ci in range(N_CORES)])
        out[f"rl{h}"] = np.stack([_wrap_slots(rl_a[h][ci], nb[h], SB)
                                  for ci in range(N_CORES)])
        out[f"val{h}"] = np.stack([_wrap_slots(val_a[h][ci], nb[h], SB)
                                   for ci in range(N_CORES)])
    return out


def _trace_kernel(nc, prog, nb, gamma_trivial, beta_trivial):
    gd = BF16 if USE_BF16_GATHER else F32
    stripes = [nb[h] * SB for h in range(2)]
    icolumns = [nb[h] * SB * 8 for h in range(2)]     # int16 idx columns

    xlo = nc.declare_dram_parameter("xlo", [XLO_ROWS, D], gd, isOutput=False)
    xhi = nc.declare_dram_parameter("xhi", [N_NODES - HI_BASE, D], gd, isOutput=False)
    xT = nc.declare_dram_parameter("xT", [D, PAD_ROWS], F32, isOutput=False)
    WselfT = nc.declare_dram_parameter("WselfT", [D, D], F32, isOutput=False)
    WinT = nc.declare_dram_parameter("WinT", [D, D], F32, isOutput=False)
    WoutT = nc.declare_dram_parameter("WoutT", [D, D], F32, isOutput=False)
    bself = nc.declare_dram_parameter("bself", [D, 1], F32, isOutput=False)
    idx_d, rl_d, val_d = [], [], []
    for h in range(2):
        idx_d.append(nc.declare_dram_parameter(f"idx{h}", [128, icolumns[h]], I16,
                                               isOutput=False))
        rl_d.append(nc.declare_dram_parameter(f"rl{h}", [128, stripes[h]], gd,
                                              isOutput=False))
        val_d.append(nc.declare_dram_parameter(f"val{h}", [128, stripes[h]], gd,
                                               isOutput=False))
    if not gamma_trivial:
        gamma_d = nc.declare_dram_parameter("gamma_rep", [128, D], F32, isOutput=False)
    if not beta_trivial:
        beta_d = nc.declare_dram_parameter("beta_rep", [128, D], F32, isOutput=False)
    out_d = nc.declare_dram_parameter("out", [PAD_ROWS, D], F32, isOutput=True)

    xtab = [xlo, xhi]

    with tile.TileContext(nc) as tc:
        with (
            tc.tile_pool(name="const", bufs=1) as constp,
            tc.tile_pool(name="g0", bufs=2) as g0pool,
            tc.tile_pool(name="g1", bufs=2) as g1pool,
            tc.tile_pool(name="meta", bufs=4) as mpool,
            tc.tile_pool(name="sbuf", bufs=3) as spool,
            tc.tile_pool(name="outp", bufs=4) as opool,
            tc.tile_pool(name="psumA", bufs=3, space="PSUM") as psA,
            tc.tile_pool(name="psumH", bufs=2, space="PSUM") as psH,
        ):
            gpool = [g0pool, g1pool]
            # ---- constants ----
            WselfT_s = constp.tile([D, D], F32, tag="wself")
            WinT_s = constp.tile([D, D], F32, tag="win")
            WoutT_s = constp.tile([D, D], F32, tag="wout")
            bself_s = constp.tile([D, 1], F32, tag="bself")
            ident = constp.tile([128, 128], F32, tag="ident")
            xT_s = constp.tile([D, PAD_ROWS], F32, tag="xt")
            iota_s = constp.tile([128, WIN], gd, tag="iota")
            nc.sync.dma_start(out=WselfT_s[:], in_=WselfT[:])
            nc.sync.dma_start(out=WinT_s[:], in_=WinT[:])
            nc.sync.dma_start(out=WoutT_s[:], in_=WoutT[:])
            nc.sync.dma_start(out=bself_s[:], in_=bself[:])
            nc.sync.dma_start(out=xT_s[:], in_=xT[:])
            make_identity(nc, ident[:])
            nc.gpsimd.iota(iota_s[:], pattern=[[1, WIN]], base=0,
                           channel_multiplier=0,
                           allow_small_or_imprecise_dtypes=True)
            if not gamma_trivial:
                gamma_s = constp.tile([128, D], F32, tag="gamma")
                nc.sync.dma_start(out=gamma_s[:], in_=gamma_d[:])
            if not beta_trivial:
                beta_s = constp.tile([128, D], F32, tag="beta")
                nc.sync.dma_start(out=beta_s[:], in_=beta_d[:])

            # ---- per-stream gather batches ----
            state = [{"batch": None, "cursor": 0}, {"batch": None, "cursor": 0}]

            def make_batch(h, bi):
                gt = gpool[h].tile([128, SB, D], gd, tag="g")
                it = mpool.tile([128, SB * 8], I16, tag=f"idx{h}")
                rt = mpool.tile([128, SB], gd, tag=f"rl{h}")
                vt = mpool.tile([128, SB], gd, tag=f"val{h}")
                St = gpool[h].tile([128, SB, WIN], gd, tag="s")
                c0 = bi * SB * 8
                nc.sync.dma_start(out=it[:], in_=idx_d[h][:, c0:c0 + SB * 8])
                nc.sync.dma_start(out=rt[:], in_=rl_d[h][:, bi * SB:(bi + 1) * SB])
                nc.sync.dma_start(out=vt[:], in_=val_d[h][:, bi * SB:(bi + 1) * SB])
                nc.gpsimd.dma_gather(
                    out_ap=gt[:], in_ap=xtab[h][:], idxs_ap=it[:],
                    num_idxs=SB * 128, num_idxs_reg=SB * 128, elem_size=D,
                    single_packet=False)
                nc.vector.tensor_tensor(
                    out=St[:],
                    in0=iota_s[:, None, :].broadcast_to([128, SB, WIN]),
                    in1=rt[:, :, None].broadcast_to([128, SB, WIN]),
                    op=mybir.AluOpType.is_equal)
                nc.vector.tensor_tensor(
                    out=St[:], in0=St[:],
                    in1=vt[:, :, None].broadcast_to([128, SB, WIN]),
                    op=mybir.AluOpType.mult)
                return gt, St

            def chunk_tiles(h):
                st = state[h]
                bi, off = divmod(st["cursor"], SB)
                if off == 0:
                    st["batch"] = make_batch(h, bi)
                st["cursor"] += 1
                gt, St = st["batch"]
                return gt[:, off, :], St[:, off, :]

            for b in range(BLOCKS):
                aggs = {}
                for (s, w, c_lo, c_hi) in prog[b]:
                    if w == 0:
                        pa = psA.tile([128, 128], F32, tag="pa", space="PSUM",
                                      name=f"pa_b{b}_s{s}")
                        agg_t = spool.tile([128, 128], F32, tag="agg",
                                           name=f"agg_b{b}_s{s}")
                        aggs[s] = (pa, agg_t)
                    pa, agg = aggs[s]
                    total = c_lo + c_hi
                    k = 0
                    for h, c in ((0, c_lo), (1, c_hi)):
                        for _ in range(c):
                            g_ap, s_ap = chunk_tiles(h)
                            nc.tensor.matmul(
                                out=pa[:, w * WIN:(w + 1) * WIN],
                                lhsT=g_ap, rhs=s_ap,
                                start=(k == 0), stop=(k == total - 1))
                            k += 1
                    if w == 1:
                        nc.scalar.copy(out=agg[:], in_=pa[:])

                ph = psH.tile([128, 128], F32, tag="ph", space="PSUM")
                nc.tensor.matmul(out=ph[:], lhsT=WselfT_s[:],
                                 rhs=xT_s[:, b * 128:(b + 1) * 128],
                                 start=True, stop=False)
                nc.tensor.matmul(out=ph[:], lhsT=WinT_s[:], rhs=aggs[0][1][:],
                                 start=False, stop=False)
                nc.tensor.matmul(out=ph[:], lhsT=WoutT_s[:], rhs=aggs[1][1][:],
                                 start=False, stop=True)
                hT = spool.tile([128, 128], F32, tag="ht")
                nc.vector.tensor_scalar(out=hT[:], in0=ph[:],
                                        scalar1=bself_s[:, :1], scalar2=None,
                                        op0=mybir.AluOpType.add)
                pt = psH.tile([128, 128], F32, tag="pt", space="PSUM")
                nc.tensor.transpose(out=pt[:], in_=hT[:], identity=ident[:])

                # layernorm over free dim + relu
                ssum = spool.tile([128, 1], F32, tag="ssum")
                nc.vector.reduce_sum(out=ssum[:], in_=pt[:],
                                     axis=mybir.AxisListType.X)
                sq = spool.tile([128, 128], F32, tag="sq")
                sqsum = spool.tile([128, 1], F32, tag="sqsum")
                nc.scalar.activation(out=sq[:], in_=pt[:],
                                     func=mybir.ActivationFunctionType.Square,
                                     accum_out=sqsum[:])
                mu = spool.tile([128, 1], F32, tag="mu")
                nc.vector.tensor_scalar_mul(out=mu[:], in0=ssum[:], scalar1=1.0 / D)
                musq = spool.tile([128, 1], F32, tag="musq")
                nc.vector.tensor_tensor(out=musq[:], in0=mu[:], in1=mu[:],
                                        op=mybir.AluOpType.mult)
                var = spool.tile([128, 1], F32, tag="var")
                nc.vector.tensor_scalar(out=var[:], in0=sqsum[:],
                                        scalar1=1.0 / D, scalar2=LN_EPS,
                                        op0=mybir.AluOpType.mult,
                                        op1=mybir.AluOpType.add)
                nc.vector.tensor_tensor(out=var[:], in0=var[:], in1=musq[:],
                                        op=mybir.AluOpType.subtract)
                std = spool.tile([128, 1], F32, tag="std")
                nc.scalar.activation(out=std[:], in_=var[:],
                                     func=mybir.ActivationFunctionType.Sqrt)
                rstd = spool.tile([128, 1], F32, tag="rstd")
                nc.vector.reciprocal(out=rstd[:], in_=std[:])
                nrm = opool.tile([128, 128], F32, tag="nrm")
                nc.vector.tensor_scalar(out=nrm[:], in0=pt[:],
                                        scalar1=mu[:, :1], scalar2=rstd[:, :1],
                                        op0=mybir.AluOpType.subtract,
                                        op1=mybir.AluOpType.mult)
                if not gamma_trivial:
                    nc.vector.tensor_tensor(out=nrm[:], in0=nrm[:], in1=gamma_s[:],
                                            op=mybir.AluOpType.mult)
                if not beta_trivial:
                    nc.vector.tensor_tensor(out=nrm[:], in0=nrm[:], in1=beta_s[:],
                                            op=mybir.AluOpType.add)
                ot = opool.tile([128, 128], F32, tag="o")
                nc.scalar.activation(out=ot[:], in_=nrm[:],
                                     func=mybir.ActivationFunctionType.Relu)
                nc.sync.dma_start(out=out_d[b * 128:(b + 1) * 128, :], in_=ot[:])


def build(x, adj_in_rows, adj_in_cols, adj_in_vals,
          adj_out_rows, adj_out_cols, adj_out_vals,
          W_self, b_self, W_in, W_out, ln_gamma, ln_beta):
    """Trace + compile; returns (nc, in_maps)."""
    x = np.asarray(x, dtype=np.float32)
    sets = [
        (np.asarray(adj_in_rows, np.int64), np.asarray(adj_in_cols, np.int64),
         np.asarray(adj_in_vals, np.float32)),
        (np.asarray(adj_out_rows, np.int64), np.asarray(adj_out_cols, np.int64),
         np.asarray(adj_out_vals, np.float32)),
    ]
    W_self = np.asarray(W_self, np.float32)
    W_in = np.asarray(W_in, np.float32)
    W_out = np.asarray(W_out, np.float32)
    b_self = np.asarray(b_self, np.float32)
    ln_gamma = np.asarray(ln_gamma, np.float32)
    ln_beta = np.asarray(ln_beta, np.float32)

    lay = _build_layout(sets)
    gamma_trivial = bool(np.all(ln_gamma == 1.0))
    beta_trivial = bool(np.all(ln_beta == 0.0))

    nc = bacc.Bacc("TRN2", target_bir_lowering=False, debug=False,
                   num_devices=N_CORES, dynamic_dma_scratch_size=81920)
    _trace_kernel(nc, lay["prog"], lay["nb"], gamma_trivial, beta_trivial)
    nc.compile()

    gdt = np.dtype(ml_dtypes.bfloat16) if USE_BF16_GATHER else np.float32
    xlo = np.ascontiguousarray(x[:XLO_ROWS]).astype(gdt)
    xhi = np.ascontiguousarray(x[HI_BASE:]).astype(gdt)
    in_maps = []
    for ci in range(N_CORES):
        r0 = ci * ROWS_PER_CORE
        xT_c = np.zeros((D, PAD_ROWS), dtype=np.float32)
        xT_c[:, :ROWS_PER_CORE] = x[r0:r0 + ROWS_PER_CORE].T
        m = {
            "xlo": xlo, "xhi": xhi, "xT": xT_c,
            "WselfT": np.ascontiguousarray(W_self.T),
            "WinT": np.ascontiguousarray(W_in.T),
            "WoutT": np.ascontiguousarray(W_out.T),
            "bself": np.ascontiguousarray(b_self[:, None]),
        }
        for h in range(2):
            m[f"idx{h}"] = lay[f"idx{h}"][ci]
            m[f"rl{h}"] = lay[f"rl{h}"][ci].astype(gdt)
            m[f"val{h}"] = lay[f"val{h}"][ci].astype(gdt)
        if not gamma_trivial:
            m["gamma_rep"] = np.tile(ln_gamma[None, :], (128, 1))
        if not beta_trivial:
            m["beta_rep"] = np.tile(ln_beta[None, :], (128, 1))
        in_maps.append(m)
    return nc, in_maps


def kernel(**inputs):
    nc, in_maps = build(**inputs)
    _split_multi_waits(nc)
    res = run_bass_kernel_spmd(nc, in_maps, core_ids=list(range(N_CORES)))
    out = np.concatenate(
        [res.results[ci]["out"][:ROWS_PER_CORE] for ci in range(N_CORES)], axis=0)
    return out.astype(np.float32)


def make_timed_runner(nc, in_maps, n_cores):
    """Jitted 8-core SPMD executable with repeat-callable timing (mirrors
    concourse.bass2jax.run_bass_via_pjrt's multi-core path)."""
    import time
    import jax
    from jax.experimental.shard_map import shard_map
    from jax.sharding import Mesh, PartitionSpec, NamedSharding
    from concourse.bass2jax import _bass_exec_p, install_neuronx_cc_hook, \
        partition_id_tensor

    install_neuronx_cc_hook()
    partition_name = nc.partition_id_tensor.name if nc.partition_id_tensor else None
    in_names, out_names, out_avals, zero_outs = [], [], [], []
    for alloc in nc.m.functions[0].allocations:
        if not isinstance(alloc, mybir.MemoryLocationSet):
            continue
        name = alloc.memorylocations[0].name
        if alloc.kind == "ExternalInput":
            if name != partition_name:
                in_names.append(name)
        elif alloc.kind == "ExternalOutput":
            shape = tuple(alloc.tensor_shape)
            dtype = mybir.dt.np(alloc.dtype)
            out_names.append(name)
            out_avals.append(jax.core.ShapedArray(shape, dtype))
            zero_outs.append(np.zeros(shape, dtype))
    n_params, n_outs = len(in_names), len(out_avals)
    all_in_names = list(in_names) + list(out_names)
    if partition_name is not None:
        all_in_names.append(partition_name)

    def _body(*args):
        operands = list(args)
        if partition_name is not None:
            operands.append(partition_id_tensor())
        return tuple(_bass_exec_p.bind(
            *operands, out_avals=tuple(out_avals), in_names=tuple(all_in_names),
            out_names=tuple(out_names), lowering_input_output_aliases=(),
            sim_require_finite=True, sim_require_nnan=True, nc=nc))

    devices = jax.devices()[:n_cores]
    mesh = Mesh(np.asarray(devices), ("core",))
    in_specs = (PartitionSpec("core"),) * (n_params + n_outs)
    out_specs = (PartitionSpec("core"),) * n_outs
    sharded = jax.jit(
        shard_map(_body, mesh=mesh, in_specs=in_specs, out_specs=out_specs,
                  check_rep=False),
        donate_argnums=tuple(range(n_params, n_params + n_outs)),
        keep_unused=True)
    shard0 = NamedSharding(mesh, PartitionSpec("core"))
    dev_in = [jax.device_put(
        np.concatenate([np.asarray(in_maps[c][nm]) for c in range(n_cores)], axis=0),
        shard0) for nm in in_names]
    concat_zeros = [np.zeros((n_cores * z.shape[0], *z.shape[1:]), z.dtype)
                    for z in zero_outs]

    def run():
        dev_zeros = [jax.device_put(a, shard0) for a in concat_zeros]
        jax.block_until_ready(dev_zeros)
        t0 = time.perf_counter()
        outs = sharded(*dev_in, *dev_zeros)
        jax.block_until_ready(outs)
        return outs, time.perf_counter() - t0

    def results(outs):
        res = []
        for c in range(n_cores):
            d = {}
            for i, nm in enumerate(out_names):
                per = np.asarray(outs[i])
                rows = per.shape[0] // n_cores
                d[nm] = per[c * rows:(c + 1) * rows]
            res.append(d)
        return res

    return run, results

